# revision 33
# baseline (speedup 1.0000x reference)
"""DGIN (D-MPNN + GIN) message-passing network on 8 Trainium2 NeuronCores.

Strategy (row-sharded 1D graph parallel, per the sharding hint):
  - Edge rows (E=8192) and node rows (N=4096) are split 8 ways; each core owns
    the matching row slices of A_e [E,E], A_ne [N,E], A [N,N] and feature rows.
  - The adjacency matrices are transposed and cast to fp8e4 ON THE HOST
    (0/1 values are exact in fp8), so the device streams A^T shards straight
    into SBUF caches with plain DMAs - no on-device transposes, no casts,
    and 4x less HBM traffic than fp32.  Bulk streams ride the ACT HWDGE ring
    so they never block latency-critical transfers on the SP ring.
  - The replicated state h ([E,64]/[N,128] bf16) is AllGathered in TWO HALVES
    per iteration; the next iteration's contraction consumes the first half's
    k-blocks before the second half lands, hiding most collective latency and
    keeping the PE clock warm.
  - Matmuls contract over the partition axis with the state stationary; the
    two 64-wide D-MPNN streams pack into PE column halves, and the pair-merge
    matmul uses (I - J/H) instead of identity, performing the LayerNorm
    mean-centering for free.  LN affine factors are folded into the adjacent
    weights on the host (g1 -> W_pass, GIN feature-centering C_G -> W_gin for
    t<2); the init bias rides an augmented ones-column.
  - identity_matrix input is mathematically unused by the reference and is
    never shipped to the device.
"""

import contextlib

import ml_dtypes
import numpy as np

import concourse.bass as bass
import concourse.mybir as mybir
import concourse.tile as tile
from concourse import bacc
from concourse.bass_utils import run_bass_kernel_spmd
from concourse.masks import make_identity

dt = mybir.dt
Alu = mybir.AluOpType
Act = mybir.ActivationFunctionType

# problem shape (hardcoded per contest contract)
E, N = 8192, 4096
NF, EFD = 64, 16
H = 64
G = NF + H  # 128
TD, TG = 3, 3
EPS_LN = 1e-6

P = 128
C = 8
ES = E // C           # 1024 edge rows per core
NS = N // C           # 512 node rows per core
KBE = E // P          # 64 contraction blocks over E
KBN = N // P          # 32 contraction blocks over N
ET = ES // P          # 8 edge row-tiles per shard
NT = NS // P          # 4 node row-tiles per shard
ETH = ET // 2         # 4 edge row-tiles per AG half
NTH = NT // 2         # 2 node row-tiles per AG half
XA = NF + EFD + 1     # 81 rows of the augmented feature transpose

# k-block id covered by slot j of an AG half buffer (h_curA/B etc.).
# The host swizzle stores the adjacency k-blocks in THIS order, so cache
# slot i == AG-half slot i and the first half of the cache serves half A.
KBA_E = [c * ET + et for c in range(C) for et in range(ETH)]
KBB_E = [c * ET + ETH + et for c in range(C) for et in range(ETH)]
KBA_N = [c * NT + nt for c in range(C) for nt in range(NTH)]
KBB_N = [c * NT + NTH + nt for c in range(C) for nt in range(NTH)]
ORD_E = KBA_E + KBB_E
ORD_N = KBA_N + KBB_N


def _build(repeat: int = 1, fake_cc: bool = False):
    nc = bacc.Bacc("TRN2", target_bir_lowering=False, debug=False,
                   enable_asserts=True, num_devices=1 if fake_cc else C)

    # ---- per-core external inputs (host pre-transposed / pre-cast and
    # pre-swizzled into SBUF tile layout: row p holds all its k-blocks) ----
    aeT_s = nc.dram_tensor("aeT_s", [P, KBE * ES], dt.float8e4,
                           kind="ExternalInput")
    aneT_s = nc.dram_tensor("aneT_s", [P, KBE * NS], dt.float8e4,
                            kind="ExternalInput")
    aT_s = nc.dram_tensor("aT_s", [P, KBN * NS], dt.float8e4,
                          kind="ExternalInput")
    xaugT_s = nc.dram_tensor("xaugT_s", [XA, ES], dt.bfloat16, kind="ExternalInput")
    waug = nc.dram_tensor("waug", [XA, H], dt.bfloat16, kind="ExternalInput")
    nf_s = nc.dram_tensor("nf_s", [P, NT * NF], dt.bfloat16, kind="ExternalInput")
    wp_t = nc.dram_tensor("wp_t", [H, TD * H], dt.bfloat16, kind="ExternalInput")
    wg_t = nc.dram_tensor("wg_t", [G, TG * G], dt.bfloat16, kind="ExternalInput")
    c1_t = nc.dram_tensor("c1_t", [1, TD * H], dt.float32, kind="ExternalInput")
    bg_t = nc.dram_tensor("bg_t", [1, TG * G], dt.float32, kind="ExternalInput")
    g2_t = nc.dram_tensor("g2_t", [1, H], dt.float32, kind="ExternalInput")
    b2_t = nc.dram_tensor("b2_t", [1, H], dt.float32, kind="ExternalInput")
    g3_t = nc.dram_tensor("g3_t", [1, G], dt.float32, kind="ExternalInput")
    b3_t = nc.dram_tensor("b3_t", [1, G], dt.float32, kind="ExternalInput")
    eps_t = nc.dram_tensor("eps_t", [1, G], dt.float32, kind="ExternalInput")
    g4_t = nc.dram_tensor("g4_t", [1, G], dt.float32, kind="ExternalInput")
    b4_t = nc.dram_tensor("b4_t", [1, G], dt.float32, kind="ExternalInput")

    out = nc.dram_tensor("out", [1, G], dt.float32, kind="ExternalOutput")

    # ---- internal DRAM bounce buffers for collectives (per repeat) ----
    def mk_ag(tag, r, rows, k, width):
        i = nc.dram_tensor(f"{tag}i_{r}", [rows, k, width], dt.bfloat16)
        o = nc.dram_tensor(f"{tag}o_{r}", [C * rows, k, width], dt.bfloat16)
        return i, o

    agh_r = [[tuple(mk_ag(f"agh{t}{hf}", r, P, ETH, H) for hf in "ab")
              for t in range(TD + 1)] for r in range(repeat)]
    agv_r = [[tuple(mk_ag(f"agv{t}{hf}", r, P, NTH, G) for hf in "ab")
              for t in range(TG)] for r in range(repeat)]
    ags_in_r = [nc.dram_tensor(f"ags_in_{r}", [1, G], dt.float32)
                for r in range(repeat)]
    ags_out_r = [nc.dram_tensor(f"ags_out_{r}", [C, G], dt.float32)
                 for r in range(repeat)]

    rg = [list(range(C))]
    drain_cnt = [0]

    def drain(out_ap, in_ap):
        """PSUM -> SBUF copy, alternating DVE / ACT to split the load."""
        drain_cnt[0] += 1
        if drain_cnt[0] % 2 == 0:
            nc.vector.tensor_copy(out_ap, in_ap)
        else:
            nc.scalar.copy(out_ap, in_ap)

    def all_gather(inp, outp):
        if fake_cc:
            nc.sync.dma_start(outp[0:inp.shape[0]], inp[:])
        else:
            nc.gpsimd.collective_compute("AllGather", Alu.bypass, replica_groups=rg,
                                         ins=[inp[:]], outs=[outp[:]])

    with tile.TileContext(nc) as tc:
      for _rep in range(repeat):
        agh = agh_r[_rep]
        agv = agv_r[_rep]
        ags_in, ags_out = ags_in_r[_rep], ags_out_r[_rep]
        stack = contextlib.ExitStack()
        const = stack.enter_context(tc.tile_pool(name="const", bufs=1))
        stream = stack.enter_context(tc.tile_pool(name="stream", bufs=3))
        psA = stack.enter_context(tc.tile_pool(name="psA", bufs=2, space="PSUM"))
        psT = stack.enter_context(tc.tile_pool(name="psT", bufs=4, space="PSUM"))
        psW = stack.enter_context(tc.tile_pool(name="psW", bufs=2, space="PSUM"))

        # ================= P0: constants / parameters =================
        ident_b = const.tile([P, P], dt.bfloat16)
        make_identity(nc, ident_b[:])
        ones_g = const.tile([P, 1], dt.bfloat16)
        nc.vector.memset(ones_g[:], 1.0)
        ones8f = const.tile([C, 1], dt.float32)
        nc.vector.memset(ones8f[:], 1.0)
        epsb = const.tile([P, 1], dt.float32)
        nc.vector.memset(epsb[:], EPS_LN)
        # sid_c = [C64; C64] with C64 = I - J/64: pair-merge + mean-centering
        sid_c = const.tile([P, H], dt.bfloat16)
        nc.gpsimd.memset(sid_c[:], 0.0)
        make_identity(nc, sid_c[0:H, :], nomemset=True)
        make_identity(nc, sid_c[H:P, :], nomemset=True)
        nc.vector.tensor_scalar_add(sid_c[:], sid_c[:], -1.0 / H)

        xaugT = const.tile([XA, ES], dt.bfloat16)
        nc.sync.dma_start(xaugT[:], xaugT_s[:])
        waug_sb = const.tile([XA, H], dt.bfloat16)
        nc.sync.dma_start(waug_sb[:], waug[:])
        wp_sb = const.tile([H, TD, H], dt.bfloat16)
        nc.sync.dma_start(wp_sb[:], wp_t[:].rearrange("a (t b) -> a t b", t=TD))
        wg_sb = const.tile([G, TG, G], dt.bfloat16)
        nc.sync.dma_start(wg_sb[:], wg_t[:].rearrange("a (t b) -> a t b", t=TG))
        nf_sb = const.tile([P, NT, NF], dt.bfloat16)
        nc.sync.dma_start(nf_sb[:], nf_s[:].rearrange("p (nt f) -> p nt f", nt=NT))

        # broadcast LN / bias rows to all 128 partitions once
        def bcast_param(src, width, name):
            row = const.tile([1, width], dt.float32, name=f"row_{name}")
            nc.sync.dma_start(row[:], src)
            full = const.tile([P, width], dt.float32, name=f"bc_{name}")
            nc.gpsimd.partition_broadcast(full[:], row[:])
            return full

        c1m = bcast_param(c1_t[:], TD * H, "c1")     # [P, TD*H]
        bgm = bcast_param(bg_t[:], TG * G, "bg")     # [P, TG*G]
        g2m = bcast_param(g2_t[:], H, "g2")
        b2m = bcast_param(b2_t[:], H, "b2")
        g3m = bcast_param(g3_t[:], G, "g3")
        b3m = bcast_param(b3_t[:], G, "b3")
        epsm = bcast_param(eps_t[:], G, "eps")
        nc.vector.tensor_scalar_add(epsm[:], epsm[:], 1.0)  # (1 + eps)
        g4_sb = const.tile([1, G], dt.float32)
        nc.sync.dma_start(g4_sb[:], g4_t[:])
        b4_sb = const.tile([1, G], dt.float32)
        nc.sync.dma_start(b4_sb[:], b4_t[:])

        # ===== adjacency caches: contiguous bulk fp8 DMA (128 descriptors
        # per transfer thanks to the host swizzle), on the ACT HWDGE ring =====
        atc_pool = stack.enter_context(tc.tile_pool(name="atc_pool", bufs=1))
        atc = atc_pool.tile([P, KBE, ES], dt.float8e4)   # A_e^T, 64 KiB/part
        atne = atc_pool.tile([P, KBE, NS], dt.float8e4)  # A_ne^T, 32 KiB/part
        atc2 = atc_pool.tile([P, KBN, NS], dt.float8e4)  # A^T, 16 KiB/part
        aeT_r = aeT_s[:].rearrange("p (kb c) -> p kb c", c=ES)
        for kc in range(8):  # chunked so t=0 matmuls can chase the DMA
            nc.scalar.dma_start(atc[:, kc * 8:(kc + 1) * 8, :],
                                aeT_r[:, kc * 8:(kc + 1) * 8, :])
        # (A_ne^T / A^T loads are issued inside the t=0 loop so their traffic
        # does not contend with the A_e^T stream the first burst chases.)

        dm = stack.enter_context(tc.tile_pool(name="dm", bufs=1))

        # =============== P1: h0 = relu([eal|ef|1] @ Waug) ===============
        h0n = dm.tile([P, ET, H], dt.bfloat16)   # h0 natural, kept all D-MPNN
        for et in range(ET):
            ps = psW.tile([P, H], dt.float32, tag="wps", name=f"h0{et}")
            nc.tensor.matmul(ps[:], xaugT[:, et * P:(et + 1) * P], waug_sb[:],
                             start=True, stop=True)
            nc.scalar.activation(h0n[:, et, :], ps[:], Act.Relu)
        nc.sync.dma_start(agh[0][0][0][:], h0n[:, 0:ETH, :])
        all_gather(agh[0][0][0], agh[0][0][1])
        nc.sync.dma_start(agh[0][1][0][:], h0n[:, ETH:ET, :])
        all_gather(agh[0][1][0], agh[0][1][1])

        # h0c_t = h0 + (b1 @ W_pass[t]) broadcast, one tile per t
        h0c = dm.tile([P, TD, ET, H], dt.bfloat16)
        for t in range(TD):
            for et in range(ET):
                nc.vector.tensor_add(h0c[:, t, et, :], h0n[:, et, :],
                                     c1m[:, t * H:t * H + H])

        # D-MPNN working tiles
        mTc = dm.tile([H, ES], dt.bfloat16)      # centered m^T
        cen = dm.tile([P, ET, H], dt.bfloat16)   # centered m natural
        var = dm.tile([P, ET], dt.float32)
        rstd = dm.tile([P, ET], dt.float32)
        sqf = dm.tile([P, ETH, H], dt.float32)   # squares staging
        mhat = dm.tile([P, ET, H], dt.bfloat16)
        mhatT = dm.tile([H, ES], dt.bfloat16)
        tadd = dm.tile([P, ET, H], dt.float32)

        def load_h(ag_pair, name):
            """Load the two AG halves into separate k-slot tiles."""
            hA = dm.tile([P, C * ETH, H], dt.bfloat16, tag="hA", bufs=2,
                         name=name + "a")
            hB = dm.tile([P, C * ETH, H], dt.bfloat16, tag="hB", bufs=2,
                         name=name + "b")
            nc.sync.dma_start(hA[:].rearrange("p (c e) h -> p c e h", c=C),
                              ag_pair[0][1][:].rearrange(
                                  "(c p) e h -> p c e h", p=P))
            nc.sync.dma_start(hB[:].rearrange("p (c e) h -> p c e h", c=C),
                              ag_pair[1][1][:].rearrange(
                                  "(c p) e h -> p c e h", p=P))
            return hA, hB

        warm_cnt = [0]

        def warm(n):
            """Keepalive matmuls that fill PE-idle AG windows so the HAM
            clock gate stays at full rate (results are never read)."""
            warm_cnt[0] += 1
            psd = psW.tile([H, 512], dt.float32, tag="wps",
                           name=f"warm{warm_cnt[0]}")
            for i in range(n):
                nc.tensor.matmul(psd[:], sid_c[:], atc[:, i % 4, 0:512],
                                 start=True, stop=True)

        hA, hB = load_h(agh[0], "h0")

        def edge_contract(ps_list, hA, hB, cache, ncols, name):
            """Accumulate (A^T-block contraction) into ps_list (one per
            512-col group), consuming the AG half A k-blocks first.  Cache
            slots are in AG order (host swizzle), and matmuls that share a
            stationary operand are emitted adjacently."""
            nh_half = len(KBA_E)  # 32 slots per half
            npair = nh_half // 2
            for half, hsrc in enumerate((hA, hB)):
                base = half * nh_half
                for i in range(npair):
                    je, jo = 2 * i, 2 * i + 1
                    first = (half == 0 and i == 0)
                    last = (half == 1 and i == npair - 1)
                    for nh, ps in enumerate(ps_list):
                        sl = slice(nh * 512, nh * 512 + ncols)
                        nc.tensor.matmul(ps[0:H, :], hsrc[:, je, :],
                                         cache[:, base + je, sl],
                                         start=first, stop=last,
                                         tile_position=(0, 0))
                    for nh, ps in enumerate(ps_list):
                        sl = slice(nh * 512, nh * 512 + ncols)
                        nc.tensor.matmul(ps[H:P, :], hsrc[:, jo, :],
                                         cache[:, base + jo, sl],
                                         start=first, stop=last,
                                         tile_position=(0, H))

        # =============== P2: D-MPNN iterations ===============
        for t in range(TD):
            ps_list = [psA.tile([P, 512], dt.float32, tag="acc",
                                name=f"mT{t}_{nh}") for nh in range(2)]
            edge_contract(ps_list, hA, hB, atc, 512, f"m{t}")
            for nh, ps in enumerate(ps_list):
                sl = slice(nh * 512, (nh + 1) * 512)
                pair = stream.tile([P, 512], dt.bfloat16, tag="pairsum",
                                   name=f"pair{t}_{nh}")
                drain(pair[:], ps[:])
                ps2 = psW.tile([H, 512], dt.float32, tag="wps",
                               name=f"mTs{t}_{nh}")
                nc.tensor.matmul(ps2[:], sid_c[:], pair[:], start=True, stop=True)
                drain(mTc[:, sl], ps2[:])
            # --- per-half tail: transpose, rstd, scale, W_pass', h0+relu ---
            # op-count-minimized: transposes / matmuls land in multi-slice
            # PSUM tiles so each drain / add / relu is ONE instruction.
            h_new = dm.tile([P, ET, H], dt.bfloat16, tag="hnew", bufs=2,
                            name=f"h_new{t}")
            for hf in range(2):
                ets = range(hf * ETH, (hf + 1) * ETH)
                esl = slice(hf * ETH, (hf + 1) * ETH)
                psh = psT.tile([P, ETH, H], dt.bfloat16, tag="pst",
                               name=f"mn{t}_{hf}")
                for i, et in enumerate(ets):
                    nc.tensor.transpose(psh[:, i, :], mTc[:, et * P:(et + 1) * P],
                                        ident_b[:H, :H])
                drain(cen[:, esl, :], psh[:])
                nc.vector.tensor_mul(sqf[:], cen[:, esl, :], cen[:, esl, :])
                nc.vector.tensor_reduce(var[:, esl], sqf[:],
                                        mybir.AxisListType.X, Alu.add)
                nc.scalar.activation(rstd[:, esl], var[:, esl], Act.Sqrt,
                                     scale=1.0 / H, bias=epsb[:])
                nc.vector.reciprocal(rstd[:, esl], rstd[:, esl])
                psh2 = psT.tile([H, ETH, P], dt.bfloat16, tag="pst",
                                name=f"lt{t}_{hf}")
                for i, et in enumerate(ets):
                    nc.vector.tensor_scalar_mul(mhat[:, et, :], cen[:, et, :],
                                                rstd[:, et:et + 1])
                    nc.tensor.transpose(psh2[:, i, :], mhat[:, et, :],
                                        ident_b[:])
                drain(mhatT[:, hf * ETH * P:(hf + 1) * ETH * P]
                      .rearrange("a (e p) -> a e p", p=P), psh2[:])
                psw = psW.tile([P, ETH, H], dt.float32, tag="wps",
                               name=f"wp{t}_{hf}")
                for i, et in enumerate(ets):
                    nc.tensor.matmul(psw[:, i, :],
                                     mhatT[:, et * P:(et + 1) * P],
                                     wp_sb[:, t, :], start=True, stop=True)
                nc.vector.tensor_add(tadd[:, esl, :], psw[:],
                                     h0c[:, t, esl, :])
                nc.scalar.activation(h_new[:, esl, :], tadd[:, esl, :],
                                     Act.Relu)
                nc.sync.dma_start(agh[t + 1][hf][0][:], h_new[:, esl, :])
                all_gather(agh[t + 1][hf][0], agh[t + 1][hf][1])
            if t == 0:
                # background streams needed from P3 on; issued here so they
                # don't fight the t=0 A_e^T chase for HBM bandwidth
                nc.scalar.dma_start(
                    atne[:], aneT_s[:].rearrange("p (kb c) -> p kb c", c=NS))
                nc.scalar.dma_start(
                    atc2[:], aT_s[:].rearrange("p (kb c) -> p kb c", c=NS))
            if t > 0:
                warm(16)
            hA, hB = load_h(agh[t + 1], f"h{t + 1}")

        # ========= P3: m_v = LN(A_ne[rows] @ h); h0_v = [nf | m_v] =========
        hv = stack.enter_context(tc.tile_pool(name="hv", bufs=1))
        h0v = hv.tile([P, NT, G], dt.bfloat16)
        psv = [psA.tile([P, NS], dt.float32, tag="acc", name="mvT")]
        edge_contract(psv, hA, hB, atne, NS, "mv")
        pairv = stream.tile([P, NS], dt.bfloat16, tag="pairsum", name="pairv")
        drain(pairv[:], psv[0][:])
        psv2 = psW.tile([H, NS], dt.float32, tag="wps", name="mvTs")
        nc.tensor.matmul(psv2[:], sid_c[:], pairv[:], start=True, stop=True)
        drain(mTc[:, :NS], psv2[:])
        # LayerNorm (g2, b2) into h0v[:, :, 64:128]; center rows; AG halves
        h0e = hv.tile([P, NT, G], dt.bfloat16)
        hc0 = hv.tile([P, NT, G], dt.bfloat16)
        muv = hv.tile([P, NT], dt.float32)
        scrg = hv.tile([P, G], dt.bfloat16)
        for hf in range(2):
            nts = range(hf * NTH, (hf + 1) * NTH)
            nsl = slice(hf * NTH, (hf + 1) * NTH)
            psh = psT.tile([P, NTH, H], dt.bfloat16, tag="pst", name=f"mv{hf}")
            for i, nt in enumerate(nts):
                nc.tensor.transpose(psh[:, i, :], mTc[:, nt * P:(nt + 1) * P],
                                    ident_b[:H, :H])
            drain(cen[:, nsl, :], psh[:])
            nc.vector.tensor_copy(h0v[:, nsl, 0:NF], nf_sb[:, nsl, :])
            nc.vector.tensor_mul(sqf[:, 0:NTH, :], cen[:, nsl, :],
                                 cen[:, nsl, :])
            nc.vector.tensor_reduce(var[:, nsl], sqf[:, 0:NTH, :],
                                    mybir.AxisListType.X, Alu.add)
            nc.scalar.activation(rstd[:, nsl], var[:, nsl], Act.Sqrt,
                                 scale=1.0 / H, bias=epsb[:])
            nc.vector.reciprocal(rstd[:, nsl], rstd[:, nsl])
            for nt in nts:
                nc.vector.scalar_tensor_tensor(mhat[:, nt, :], cen[:, nt, :],
                                               rstd[:, nt:nt + 1], g2m[:],
                                               Alu.mult, Alu.mult)
                nc.vector.tensor_add(h0v[:, nt, NF:G], mhat[:, nt, :], b2m[:])
            nc.vector.tensor_reduce(muv[:, nsl], h0v[:, nsl, :],
                                    mybir.AxisListType.X, Alu.add)
            nc.scalar.mul(muv[:, nsl], muv[:, nsl], 1.0 / G)
            for nt in nts:
                nc.vector.tensor_scalar_sub(hc0[:, nt, :], h0v[:, nt, :],
                                            muv[:, nt:nt + 1])
            nc.sync.dma_start(agv[0][hf][0][:], hc0[:, nsl, :])
            all_gather(agv[0][hf][0], agv[0][hf][1])
        # h0e = (1+eps) * h0_v + b3 (matmul operand); fills the AG gap
        for nt in range(NT):
            nc.vector.tensor_mul(scrg[:], h0v[:, nt, :], epsm[:])
            nc.vector.tensor_add(h0e[:, nt, :], scrg[:], b3m[:])

        def load_hv(ag_pair, name):
            vA = hv.tile([P, C * NTH, G], dt.bfloat16, tag="vA", bufs=2,
                         name=name + "a")
            vB = hv.tile([P, C * NTH, G], dt.bfloat16, tag="vB", bufs=2,
                         name=name + "b")
            nc.sync.dma_start(vA[:].rearrange("p (c e) g -> p c e g", c=C),
                              ag_pair[0][1][:].rearrange(
                                  "(c p) e g -> p c e g", p=P))
            nc.sync.dma_start(vB[:].rearrange("p (c e) g -> p c e g", c=C),
                              ag_pair[1][1][:].rearrange(
                                  "(c p) e g -> p c e g", p=P))
            return vA, vB

        warm(14)
        vA, vB = load_hv(agv[0], "v0")

        # =============== P4: GIN iterations ===============
        zT = hv.tile([G, NS], dt.bfloat16)
        zc = hv.tile([P, NT, G], dt.bfloat16)
        pre = hv.tile([P, NT, G], dt.bfloat16)
        preT = hv.tile([G, NS], dt.bfloat16)
        varz = hv.tile([P, NT], dt.float32)
        rstdz = hv.tile([P, NT], dt.float32)
        sqg = hv.tile([P, NTH, G], dt.float32)
        lnp = hv.tile([P, G], dt.bfloat16)

        hv_final = None
        for t in range(TG):
            psz = psA.tile([G, NS], dt.float32, tag="acc", name=f"zT{t}")
            nv = len(KBA_N)
            for half, vsrc in enumerate((vA, vB)):
                for j in range(nv):
                    nc.tensor.matmul(psz[:], vsrc[:, j, :],
                                     atc2[:, half * nv + j, :],
                                     start=(half == 0 and j == 0),
                                     stop=(half == 1 and j == nv - 1))
            drain(zT[:], psz[:])
            hv_new = hv.tile([P, NT, G], dt.bfloat16, tag="hvnew", bufs=2,
                             name=f"hv_new{t}")
            for hf in range(2):
                nts = range(hf * NTH, (hf + 1) * NTH)
                nsl = slice(hf * NTH, (hf + 1) * NTH)
                psh = psT.tile([P, NTH, G], dt.bfloat16, tag="pst",
                               name=f"zn{t}_{hf}")
                for i, nt in enumerate(nts):
                    nc.tensor.transpose(psh[:, i, :],
                                        zT[:, nt * P:(nt + 1) * P], ident_b[:])
                drain(zc[:, nsl, :], psh[:])
                nc.vector.tensor_mul(sqg[:], zc[:, nsl, :], zc[:, nsl, :])
                nc.vector.tensor_reduce(varz[:, nsl], sqg[:],
                                        mybir.AxisListType.X, Alu.add)
                nc.scalar.activation(rstdz[:, nsl], varz[:, nsl], Act.Sqrt,
                                     scale=1.0 / G, bias=epsb[:])
                nc.vector.reciprocal(rstdz[:, nsl], rstdz[:, nsl])
                psh2 = psT.tile([P, NTH, P], dt.bfloat16, tag="pst",
                                name=f"pT{t}_{hf}")
                for i, nt in enumerate(nts):
                    nc.vector.scalar_tensor_tensor(lnp[:], zc[:, nt, :],
                                                   rstdz[:, nt:nt + 1], g3m[:],
                                                   Alu.mult, Alu.mult)
                    nc.vector.tensor_add(pre[:, nt, :], lnp[:], h0e[:, nt, :])
                    nc.tensor.transpose(psh2[:, i, :], pre[:, nt, :],
                                        ident_b[:])
                drain(preT[:, hf * NTH * P:(hf + 1) * NTH * P]
                      .rearrange("a (e p) -> a e p", p=P), psh2[:])
                psw = psW.tile([P, NTH, G], dt.float32, tag="wps",
                               name=f"wg{t}_{hf}")
                for i, nt in enumerate(nts):
                    nc.tensor.matmul(psw[:, i, :],
                                     preT[:, nt * P:(nt + 1) * P],
                                     wg_sb[:, t, :], start=True, stop=True)
                for i, nt in enumerate(nts):
                    nc.vector.tensor_add(hv_new[:, nt, :], psw[:, i, :],
                                         bgm[:, t * G:t * G + G])
                if t < TG - 1:
                    nc.sync.dma_start(agv[t + 1][hf][0][:], hv_new[:, nsl, :])
                    all_gather(agv[t + 1][hf][0], agv[t + 1][hf][1])
            if t < TG - 1:
                warm(12)
                vA, vB = load_hv(agv[t + 1], f"v{t + 1}")
            else:
                hv_final = hv_new

        # =============== P5: readout + final LayerNorm ===============
        ps_sum = psW.tile([1, G], dt.float32, tag="wps", name="ps_sum")
        for nt in range(NT):
            nc.tensor.matmul(ps_sum[:], ones_g[:], hv_final[:, nt, :],
                             start=(nt == 0), stop=(nt == NT - 1))
        sum_sb = hv.tile([1, G], dt.float32)
        nc.vector.tensor_copy(sum_sb[:], ps_sum[:])
        nc.sync.dma_start(ags_in[:], sum_sb[:])
        if fake_cc:
            nc.sync.dma_start(ags_out[0:1], ags_in[:])
        else:
            nc.gpsimd.collective_compute("AllGather", Alu.bypass,
                                         replica_groups=rg,
                                         ins=[ags_in[:]], outs=[ags_out[:]])
        gall = hv.tile([C, G], dt.float32)
        nc.sync.dma_start(gall[:], ags_out[:])
        ps_g = psW.tile([1, G], dt.float32, tag="wps", name="ps_g")
        nc.tensor.matmul(ps_g[:], ones8f[:], gall[:], start=True, stop=True)
        gsum = hv.tile([1, G], dt.float32)
        nc.vector.tensor_copy(gsum[:], ps_g[:])

        s1 = hv.tile([1, 1], dt.float32)
        nc.vector.tensor_reduce(s1[:], gsum[:], mybir.AxisListType.X, Alu.add)
        nc.scalar.mul(s1[:], s1[:], 1.0 / G)
        cenf = hv.tile([1, G], dt.float32)
        nc.vector.tensor_scalar_sub(cenf[:], gsum[:], s1[:])
        varf = hv.tile([1, 1], dt.float32)
        outf = hv.tile([1, G], dt.float32)
        nc.scalar.activation(outf[:], cenf[:], Act.Square, accum_out=varf[:])
        nc.scalar.activation(varf[:], varf[:], Act.Sqrt, scale=1.0 / G,
                             bias=epsb[:1, :])
        nc.vector.reciprocal(varf[:], varf[:])
        nc.vector.tensor_scalar_mul(cenf[:], cenf[:], varf[:])
        nc.vector.tensor_mul(outf[:], cenf[:], g4_sb[:])
        nc.vector.tensor_add(outf[:], outf[:], b4_sb[:])
        nc.sync.dma_start(out[:], outf[:])

        stack.close()
    nc.compile()
    return nc


_NC_CACHE = {}


def _get_nc(repeat: int = 1, fake_cc: bool = False):
    key = (repeat, fake_cc)
    if key not in _NC_CACHE:
        _NC_CACHE[key] = _build(repeat, fake_cc)
    return _NC_CACHE[key]


def _shard_inputs(inputs):
    f32 = np.float32
    bf16 = ml_dtypes.bfloat16
    f8 = ml_dtypes.float8_e4m3
    ae = np.asarray(inputs["adj_matrix_edges_wo"], f32)
    ane = np.asarray(inputs["atm_dir_edge_adj_matrix"], f32)
    a = np.asarray(inputs["adj_matrix"], f32)
    eal = np.asarray(inputs["edge_aligned_node_features"], f32)
    ef = np.asarray(inputs["dir_edge_features"], f32)
    nf = np.asarray(inputs["node_features"], f32)
    w_init = np.asarray(inputs["W_init"], f32)
    b_init = np.asarray(inputs["b_init"], f32)
    w_pass = np.asarray(inputs["W_pass"], f32)
    w_gin = np.asarray(inputs["W_gin"], f32)
    b_gin = np.asarray(inputs["b_gin"], f32)
    eps = np.asarray(inputs["eps"], f32)
    g1, b1 = np.asarray(inputs["g1"], f32), np.asarray(inputs["b1"], f32)
    g2, b2 = np.asarray(inputs["g2"], f32), np.asarray(inputs["b2"], f32)
    g3, b3 = np.asarray(inputs["g3"], f32), np.asarray(inputs["b3"], f32)
    g4, b4 = np.asarray(inputs["g4"], f32), np.asarray(inputs["b4"], f32)

    # host-side transposes + fp8 cast (0/1 exact in fp8e4)
    aeT = np.ascontiguousarray(ae.T).astype(f8)
    aneT = np.ascontiguousarray(ane.T).astype(f8)
    aT = np.ascontiguousarray(a.T).astype(f8)

    def swizzle(mT, cols, order):
        # [K*P, cols] -> [P, K*cols]: row p holds its k-blocks contiguously
        # (one DMA descriptor per partition), permuted into AG-half order so
        # cache slot i pairs with AG-half state slot i
        k = mT.shape[0] // P
        blocks = mT.reshape(k, P, cols)[np.asarray(order)]
        return np.ascontiguousarray(
            blocks.transpose(1, 0, 2).reshape(P, k * cols))

    # augmented init weights: h0 = relu([eal|ef|1] @ [W_init; b_init])
    waug = np.concatenate([w_init, b_init[None, :]], axis=0).astype(bf16)

    # fold LN1 affine into W_pass: LN(m)@W = mhat@(g1*W) + (b1@W)
    wp = (g1[:, None] * w_pass).transpose(1, 0, 2).reshape(H, TD * H)
    c1 = np.einsum("h,thk->tk", b1, w_pass).reshape(1, TD * H)

    # GIN: fold g3 via DVE; fold feature-centering C_G into W_gin for t<2
    # (so the carried state is always column-centered), keep t=2 plain.
    # b3 rides inside the matmul operand (h0e = (1+eps)*h0_v + b3), so the
    # post-matmul bias is b_gin alone.
    CG = np.eye(G, dtype=f32) - 1.0 / G
    wg_list, bg_list = [], []
    for t in range(TG):
        w_eff = w_gin[t] @ CG if t < TG - 1 else w_gin[t]
        b_eff = b_gin[t] @ CG if t < TG - 1 else b_gin[t]
        wg_list.append(w_eff)
        bg_list.append(b_eff)
    wg = np.stack(wg_list).transpose(1, 0, 2).reshape(G, TG * G)
    bg = np.stack(bg_list).reshape(1, TG * G)

    shared = {
        "waug": np.ascontiguousarray(waug),
        "wp_t": np.ascontiguousarray(wp.astype(bf16)),
        "wg_t": np.ascontiguousarray(wg.astype(bf16)),
        "c1_t": np.ascontiguousarray(c1),
        "bg_t": np.ascontiguousarray(bg),
        "g2_t": g2.reshape(1, H).copy(), "b2_t": b2.reshape(1, H).copy(),
        "g3_t": g3.reshape(1, G).copy(), "b3_t": b3.reshape(1, G).copy(),
        "eps_t": eps.reshape(1, G).copy(),
        "g4_t": g4.reshape(1, G).copy(), "b4_t": b4.reshape(1, G).copy(),
    }
    in_maps = []
    ones_col = np.ones((ES, 1), f32)
    for c in range(C):
        er = slice(c * ES, (c + 1) * ES)
        nr = slice(c * NS, (c + 1) * NS)
        m = dict(shared)
        m["aeT_s"] = swizzle(np.ascontiguousarray(aeT[:, er]), ES, ORD_E)
        m["aneT_s"] = swizzle(np.ascontiguousarray(aneT[:, nr]), NS, ORD_E)
        m["aT_s"] = swizzle(np.ascontiguousarray(aT[:, nr]), NS, ORD_N)
        xaug = np.concatenate([eal[er], ef[er], ones_col], axis=1)  # [ES, 81]
        m["xaugT_s"] = np.ascontiguousarray(xaug.T.astype(bf16))
        m["nf_s"] = np.ascontiguousarray(
            nf[nr].reshape(NT, P, NF).transpose(1, 0, 2).reshape(P, NT * NF)
            .astype(bf16))
        in_maps.append(m)
    return in_maps


def run(inputs, **spmd_kwargs):
    """Run on hardware; returns (output, BassKernelResults)."""
    nc = _get_nc()
    in_maps = _shard_inputs(inputs)
    res = run_bass_kernel_spmd(nc, in_maps, core_ids=list(range(C)), **spmd_kwargs)
    return res.results[0]["out"], res


def kernel(**inputs) -> np.ndarray:
    out, _ = run(inputs)
    return np.ascontiguousarray(out, dtype=np.float32)


# revision 40
# speedup vs baseline: 1.0839x; 1.0839x over previous
"""DGIN (D-MPNN + GIN) message-passing network on 8 Trainium2 NeuronCores.

Strategy (row-sharded 1D graph parallel, per the sharding hint):
  - Edge rows (E=8192) and node rows (N=4096) are split 8 ways; each core owns
    the matching row slices of A_e [E,E], A_ne [N,E], A [N,N] and feature rows.
  - The adjacency matrices are transposed and cast to fp8e4 ON THE HOST
    (0/1 values are exact in fp8), so the device streams A^T shards straight
    into SBUF caches with plain DMAs - no on-device transposes, no casts,
    and 4x less HBM traffic than fp32.  Bulk streams ride the ACT HWDGE ring
    so they never block latency-critical transfers on the SP ring.
  - The replicated state h ([E,64]/[N,128] bf16) is AllGathered in TWO HALVES
    per iteration; the next iteration's contraction consumes the first half's
    k-blocks before the second half lands, hiding most collective latency and
    keeping the PE clock warm.
  - Matmuls contract over the partition axis with the state stationary; the
    two 64-wide D-MPNN streams pack into PE column halves, and the pair-merge
    matmul uses (I - J/H) instead of identity, performing the LayerNorm
    mean-centering for free.  LN affine factors are folded into the adjacent
    weights on the host (g1 -> W_pass, GIN feature-centering C_G -> W_gin for
    t<2); the init bias rides an augmented ones-column.
  - identity_matrix input is mathematically unused by the reference and is
    never shipped to the device.
"""

import contextlib

import ml_dtypes
import numpy as np

import concourse.bass as bass
import concourse.mybir as mybir
import concourse.tile as tile
from concourse import bacc
from concourse.bass_utils import run_bass_kernel_spmd
from concourse.masks import make_identity

dt = mybir.dt
Alu = mybir.AluOpType
Act = mybir.ActivationFunctionType

# problem shape (hardcoded per contest contract)
E, N = 8192, 4096
NF, EFD = 64, 16
H = 64
G = NF + H  # 128
TD, TG = 3, 3
EPS_LN = 1e-6

P = 128
C = 8
ES = E // C           # 1024 edge rows per core
NS = N // C           # 512 node rows per core
KBE = E // P          # 64 contraction blocks over E
KBN = N // P          # 32 contraction blocks over N
ET = ES // P          # 8 edge row-tiles per shard
NT = NS // P          # 4 node row-tiles per shard
ETH = ET // 2         # 4 edge row-tiles per AG half
NTH = NT // 2         # 2 node row-tiles per AG half
XA = NF + EFD + 1     # 81 rows of the augmented feature transpose

# k-block id covered by slot j of an AG chunk buffer.  The host swizzle
# stores the adjacency k-blocks in THIS order, so cache slot i == AG slot i.
# D-MPNN h is gathered in 4 quarters (2 row-tiles each); GIN h_v in 2 halves.
NQ = 4                # AG chunks for the edge state
ETQ = ET // NQ        # 2 edge row-tiles per AG quarter
KBQ_E = [[c * ET + q * ETQ + e for c in range(C) for e in range(ETQ)]
         for q in range(NQ)]
KBA_N = [c * NT + nt for c in range(C) for nt in range(NTH)]
KBB_N = [c * NT + NTH + nt for c in range(C) for nt in range(NTH)]
ORD_E = [kb for q in range(NQ) for kb in KBQ_E[q]]
ORD_N = KBA_N + KBB_N


def _build(repeat: int = 1, fake_cc: bool = False):
    nc = bacc.Bacc("TRN2", target_bir_lowering=False, debug=False,
                   enable_asserts=True, num_devices=1 if fake_cc else C)

    # ---- per-core external inputs (host pre-transposed / pre-cast and
    # pre-swizzled into SBUF tile layout: row p holds all its k-blocks) ----
    aeT_s = nc.dram_tensor("aeT_s", [P, KBE * ES], dt.float8e4,
                           kind="ExternalInput")
    aneT_s = nc.dram_tensor("aneT_s", [P, KBE * NS], dt.float8e4,
                            kind="ExternalInput")
    aT_s = nc.dram_tensor("aT_s", [P, KBN * NS], dt.float8e4,
                          kind="ExternalInput")
    xaugT_s = nc.dram_tensor("xaugT_s", [XA, ES], dt.bfloat16, kind="ExternalInput")
    waug = nc.dram_tensor("waug", [XA, H], dt.bfloat16, kind="ExternalInput")
    nf_s = nc.dram_tensor("nf_s", [P, NT * NF], dt.bfloat16, kind="ExternalInput")
    wp_t = nc.dram_tensor("wp_t", [H, TD * H], dt.bfloat16, kind="ExternalInput")
    wg_t = nc.dram_tensor("wg_t", [G, TG * G], dt.bfloat16, kind="ExternalInput")
    c1_t = nc.dram_tensor("c1_t", [1, TD * H], dt.float32, kind="ExternalInput")
    bg_t = nc.dram_tensor("bg_t", [1, TG * G], dt.float32, kind="ExternalInput")
    g2_t = nc.dram_tensor("g2_t", [1, H], dt.float32, kind="ExternalInput")
    b2_t = nc.dram_tensor("b2_t", [1, H], dt.float32, kind="ExternalInput")
    g3_t = nc.dram_tensor("g3_t", [1, G], dt.float32, kind="ExternalInput")
    b3_t = nc.dram_tensor("b3_t", [1, G], dt.float32, kind="ExternalInput")
    eps_t = nc.dram_tensor("eps_t", [1, G], dt.float32, kind="ExternalInput")
    g4_t = nc.dram_tensor("g4_t", [1, G], dt.float32, kind="ExternalInput")
    b4_t = nc.dram_tensor("b4_t", [1, G], dt.float32, kind="ExternalInput")

    out = nc.dram_tensor("out", [1, G], dt.float32, kind="ExternalOutput")

    # ---- internal DRAM bounce buffers for collectives (per repeat) ----
    def mk_ag(tag, r, rows, k, width):
        i = nc.dram_tensor(f"{tag}i_{r}", [rows, k, width], dt.bfloat16)
        o = nc.dram_tensor(f"{tag}o_{r}", [C * rows, k, width], dt.bfloat16)
        return i, o

    agh_r = [[tuple(mk_ag(f"agh{t}q{q}", r, P, ETQ, H) for q in range(NQ))
              for t in range(TD + 1)] for r in range(repeat)]
    agv_r = [[tuple(mk_ag(f"agv{t}{hf}", r, P, NTH, G) for hf in "ab")
              for t in range(TG)] for r in range(repeat)]
    ags_in_r = [nc.dram_tensor(f"ags_in_{r}", [1, G], dt.float32)
                for r in range(repeat)]
    ags_out_r = [nc.dram_tensor(f"ags_out_{r}", [C, G], dt.float32)
                 for r in range(repeat)]

    rg = [list(range(C))]
    drain_cnt = [0]

    def drain(out_ap, in_ap):
        """PSUM -> SBUF copy, alternating DVE / ACT to split the load."""
        drain_cnt[0] += 1
        if drain_cnt[0] % 2 == 0:
            nc.vector.tensor_copy(out_ap, in_ap)
        else:
            nc.scalar.copy(out_ap, in_ap)

    def all_gather(inp, outp):
        if fake_cc:
            nc.sync.dma_start(outp[0:inp.shape[0]], inp[:])
        else:
            nc.gpsimd.collective_compute("AllGather", Alu.bypass, replica_groups=rg,
                                         ins=[inp[:]], outs=[outp[:]])

    with tile.TileContext(nc) as tc:
      for _rep in range(repeat):
        agh = agh_r[_rep]
        agv = agv_r[_rep]
        ags_in, ags_out = ags_in_r[_rep], ags_out_r[_rep]
        stack = contextlib.ExitStack()
        const = stack.enter_context(tc.tile_pool(name="const", bufs=1))
        stream = stack.enter_context(tc.tile_pool(name="stream", bufs=3))
        psA = stack.enter_context(tc.tile_pool(name="psA", bufs=2, space="PSUM"))
        psT = stack.enter_context(tc.tile_pool(name="psT", bufs=4, space="PSUM"))
        psW = stack.enter_context(tc.tile_pool(name="psW", bufs=2, space="PSUM"))

        # ================= P0: constants / parameters =================
        ident_b = const.tile([P, P], dt.bfloat16)
        make_identity(nc, ident_b[:])
        ones_g = const.tile([P, 1], dt.bfloat16)
        nc.vector.memset(ones_g[:], 1.0)
        ones8f = const.tile([C, 1], dt.float32)
        nc.vector.memset(ones8f[:], 1.0)
        epsb = const.tile([P, 1], dt.float32)
        nc.vector.memset(epsb[:], EPS_LN)
        # sid_c = [C64; C64] with C64 = I - J/64: pair-merge + mean-centering
        sid_c = const.tile([P, H], dt.bfloat16)
        nc.gpsimd.memset(sid_c[:], 0.0)
        make_identity(nc, sid_c[0:H, :], nomemset=True)
        make_identity(nc, sid_c[H:P, :], nomemset=True)
        nc.vector.tensor_scalar_add(sid_c[:], sid_c[:], -1.0 / H)

        xaugT = const.tile([XA, ES], dt.bfloat16)
        nc.sync.dma_start(xaugT[:], xaugT_s[:])
        waug_sb = const.tile([XA, H], dt.bfloat16)
        nc.sync.dma_start(waug_sb[:], waug[:])
        wp_sb = const.tile([H, TD, H], dt.bfloat16)
        nc.sync.dma_start(wp_sb[:], wp_t[:].rearrange("a (t b) -> a t b", t=TD))
        wg_sb = const.tile([G, TG, G], dt.bfloat16)
        nc.sync.dma_start(wg_sb[:], wg_t[:].rearrange("a (t b) -> a t b", t=TG))
        nf_sb = const.tile([P, NT, NF], dt.bfloat16)
        nc.sync.dma_start(nf_sb[:], nf_s[:].rearrange("p (nt f) -> p nt f", nt=NT))

        # broadcast LN / bias rows to all 128 partitions once
        def bcast_param(src, width, name):
            row = const.tile([1, width], dt.float32, name=f"row_{name}")
            nc.sync.dma_start(row[:], src)
            full = const.tile([P, width], dt.float32, name=f"bc_{name}")
            nc.gpsimd.partition_broadcast(full[:], row[:])
            return full

        c1m = bcast_param(c1_t[:], TD * H, "c1")     # [P, TD*H]
        bgm = bcast_param(bg_t[:], TG * G, "bg")     # [P, TG*G]
        g2m = bcast_param(g2_t[:], H, "g2")
        b2m = bcast_param(b2_t[:], H, "b2")
        g3m = bcast_param(g3_t[:], G, "g3")
        b3m = bcast_param(b3_t[:], G, "b3")
        epsm = bcast_param(eps_t[:], G, "eps")
        nc.vector.tensor_scalar_add(epsm[:], epsm[:], 1.0)  # (1 + eps)
        g4_sb = const.tile([1, G], dt.float32)
        nc.sync.dma_start(g4_sb[:], g4_t[:])
        b4_sb = const.tile([1, G], dt.float32)
        nc.sync.dma_start(b4_sb[:], b4_t[:])

        # ===== adjacency caches: contiguous bulk fp8 DMA (128 descriptors
        # per transfer thanks to the host swizzle), on the ACT HWDGE ring =====
        atc_pool = stack.enter_context(tc.tile_pool(name="atc_pool", bufs=1))
        atc = atc_pool.tile([P, KBE, ES], dt.float8e4)   # A_e^T, 64 KiB/part
        atne = atc_pool.tile([P, KBE, NS], dt.float8e4)  # A_ne^T, 32 KiB/part
        atc2 = atc_pool.tile([P, KBN, NS], dt.float8e4)  # A^T, 16 KiB/part
        aeT_r = aeT_s[:].rearrange("p (kb c) -> p kb c", c=ES)
        for kc in range(8):  # chunked so t=0 matmuls can chase the DMA
            nc.scalar.dma_start(atc[:, kc * 8:(kc + 1) * 8, :],
                                aeT_r[:, kc * 8:(kc + 1) * 8, :])
        # (A_ne^T / A^T loads are issued inside the t=0 loop so their traffic
        # does not contend with the A_e^T stream the first burst chases.)

        dm = stack.enter_context(tc.tile_pool(name="dm", bufs=1))

        # =============== P1: h0 = relu([eal|ef|1] @ Waug) ===============
        h0n = dm.tile([P, ET, H], dt.bfloat16)   # h0 natural, kept all D-MPNN
        for et in range(ET):
            ps = psW.tile([P, H], dt.float32, tag="wps", name=f"h0{et}")
            nc.tensor.matmul(ps[:], xaugT[:, et * P:(et + 1) * P], waug_sb[:],
                             start=True, stop=True)
            nc.scalar.activation(h0n[:, et, :], ps[:], Act.Relu)
        for q in range(NQ):
            nc.sync.dma_start(agh[0][q][0][:],
                              h0n[:, q * ETQ:(q + 1) * ETQ, :])
            all_gather(agh[0][q][0], agh[0][q][1])

        # h0c_t = h0 + (b1 @ W_pass[t]) broadcast, one tile per t
        h0c = dm.tile([P, TD, ET, H], dt.bfloat16)
        for t in range(TD):
            for et in range(ET):
                nc.vector.tensor_add(h0c[:, t, et, :], h0n[:, et, :],
                                     c1m[:, t * H:t * H + H])

        # D-MPNN working tiles
        mTc = dm.tile([H, ES], dt.bfloat16)      # centered m^T
        cen = dm.tile([P, ET, H], dt.bfloat16)   # centered m natural
        var = dm.tile([P, ET], dt.float32)
        rstd = dm.tile([P, ET], dt.float32)
        sqf = dm.tile([P, ETH, H], dt.float32)   # squares staging
        mhat = dm.tile([P, ET, H], dt.bfloat16)
        mhatT = dm.tile([H, ES], dt.bfloat16)
        tadd = dm.tile([P, ET, H], dt.float32)

        def load_h(ag_set, name):
            """Load the AG quarters into separate k-slot tiles."""
            hq = []
            for q in range(NQ):
                t_ = dm.tile([P, C * ETQ, H], dt.bfloat16, tag=f"hq{q}",
                             bufs=2, name=f"{name}q{q}")
                nc.sync.dma_start(t_[:].rearrange("p (c e) h -> p c e h", c=C),
                                  ag_set[q][1][:].rearrange(
                                      "(c p) e h -> p c e h", p=P))
                hq.append(t_)
            return hq

        warm_cnt = [0]

        def warm(n):
            """Keepalive matmuls that fill PE-idle AG windows so the HAM
            clock gate stays at full rate (results are never read)."""
            warm_cnt[0] += 1
            psd = psW.tile([H, 512], dt.float32, tag="wps",
                           name=f"warm{warm_cnt[0]}")
            for i in range(n):
                nc.tensor.matmul(psd[:], sid_c[:], atc[:, i % 4, 0:512],
                                 start=True, stop=True)

        hq = load_h(agh[0], "h0")

        def edge_contract(ps_list, hq, cache, ncols, name):
            """Accumulate (A^T-block contraction) into ps_list (one per
            512-col group), consuming AG quarters in arrival order.  Cache
            slots are in AG order (host swizzle), and matmuls that share a
            stationary operand are emitted adjacently."""
            nslot = C * ETQ  # 16 slots per quarter
            npair = nslot // 2
            for qi, hsrc in enumerate(hq):
                base = qi * nslot
                for i in range(npair):
                    je, jo = 2 * i, 2 * i + 1
                    first = (qi == 0 and i == 0)
                    last = (qi == len(hq) - 1 and i == npair - 1)
                    for nh, ps in enumerate(ps_list):
                        sl = slice(nh * 512, nh * 512 + ncols)
                        nc.tensor.matmul(ps[0:H, :], hsrc[:, je, :],
                                         cache[:, base + je, sl],
                                         start=first, stop=last,
                                         tile_position=(0, 0))
                    for nh, ps in enumerate(ps_list):
                        sl = slice(nh * 512, nh * 512 + ncols)
                        nc.tensor.matmul(ps[H:P, :], hsrc[:, jo, :],
                                         cache[:, base + jo, sl],
                                         start=first, stop=last,
                                         tile_position=(0, H))

        # =============== P2: D-MPNN iterations ===============
        for t in range(TD):
            ps_list = [psA.tile([P, 512], dt.float32, tag="acc",
                                name=f"mT{t}_{nh}") for nh in range(2)]
            edge_contract(ps_list, hq, atc, 512, f"m{t}")
            for nh, ps in enumerate(ps_list):
                sl = slice(nh * 512, (nh + 1) * 512)
                pair = stream.tile([P, 512], dt.bfloat16, tag="pairsum",
                                   name=f"pair{t}_{nh}")
                drain(pair[:], ps[:])
                ps2 = psW.tile([H, 512], dt.float32, tag="wps",
                               name=f"mTs{t}_{nh}")
                nc.tensor.matmul(ps2[:], sid_c[:], pair[:], start=True, stop=True)
                drain(mTc[:, sl], ps2[:])
            # --- per-quarter tail: transpose, rstd, scale, W_pass', h0+relu,
            # store + AllGather.  Op-count-minimized: transposes / matmuls
            # land in multi-slice PSUM tiles so each drain / add / relu is
            # ONE instruction; the earliest quarter's AG flies while later
            # quarters still compute. ---
            h_new = dm.tile([P, ET, H], dt.bfloat16, tag="hnew", bufs=2,
                            name=f"h_new{t}")
            for qf in range(NQ):
                ets = range(qf * ETQ, (qf + 1) * ETQ)
                esl = slice(qf * ETQ, (qf + 1) * ETQ)
                psh = psT.tile([P, ETQ, H], dt.bfloat16, tag="pst",
                               name=f"mn{t}_{qf}")
                for i, et in enumerate(ets):
                    nc.tensor.transpose(psh[:, i, :], mTc[:, et * P:(et + 1) * P],
                                        ident_b[:H, :H])
                drain(cen[:, esl, :], psh[:])
                nc.vector.tensor_mul(sqf[:, 0:ETQ, :], cen[:, esl, :],
                                     cen[:, esl, :])
                nc.vector.tensor_reduce(var[:, esl], sqf[:, 0:ETQ, :],
                                        mybir.AxisListType.X, Alu.add)
                nc.scalar.activation(rstd[:, esl], var[:, esl], Act.Sqrt,
                                     scale=1.0 / H, bias=epsb[:])
                nc.vector.reciprocal(rstd[:, esl], rstd[:, esl])
                psh2 = psT.tile([H, ETQ, P], dt.bfloat16, tag="pst",
                                name=f"lt{t}_{qf}")
                for i, et in enumerate(ets):
                    nc.vector.tensor_scalar_mul(mhat[:, et, :], cen[:, et, :],
                                                rstd[:, et:et + 1])
                    nc.tensor.transpose(psh2[:, i, :], mhat[:, et, :],
                                        ident_b[:])
                drain(mhatT[:, qf * ETQ * P:(qf + 1) * ETQ * P]
                      .rearrange("a (e p) -> a e p", p=P), psh2[:])
                psw = psW.tile([P, ETQ, H], dt.float32, tag="wps",
                               name=f"wp{t}_{qf}")
                for i, et in enumerate(ets):
                    nc.tensor.matmul(psw[:, i, :],
                                     mhatT[:, et * P:(et + 1) * P],
                                     wp_sb[:, t, :], start=True, stop=True)
                nc.vector.tensor_add(tadd[:, esl, :], psw[:],
                                     h0c[:, t, esl, :])
                nc.scalar.activation(h_new[:, esl, :], tadd[:, esl, :],
                                     Act.Relu)
                nc.sync.dma_start(agh[t + 1][qf][0][:], h_new[:, esl, :])
                all_gather(agh[t + 1][qf][0], agh[t + 1][qf][1])
            if t == 0:
                # background streams needed from P3 on; issued here so they
                # don't fight the t=0 A_e^T chase for HBM bandwidth
                nc.scalar.dma_start(
                    atne[:], aneT_s[:].rearrange("p (kb c) -> p kb c", c=NS))
                nc.scalar.dma_start(
                    atc2[:], aT_s[:].rearrange("p (kb c) -> p kb c", c=NS))
            if t > 0:
                warm(10)
            hq = load_h(agh[t + 1], f"h{t + 1}")

        # ========= P3: m_v = LN(A_ne[rows] @ h); h0_v = [nf | m_v] =========
        hv = stack.enter_context(tc.tile_pool(name="hv", bufs=1))
        h0v = hv.tile([P, NT, G], dt.bfloat16)
        psv = [psA.tile([P, NS], dt.float32, tag="acc", name="mvT")]
        edge_contract(psv, hq, atne, NS, "mv")
        pairv = stream.tile([P, NS], dt.bfloat16, tag="pairsum", name="pairv")
        drain(pairv[:], psv[0][:])
        psv2 = psW.tile([H, NS], dt.float32, tag="wps", name="mvTs")
        nc.tensor.matmul(psv2[:], sid_c[:], pairv[:], start=True, stop=True)
        drain(mTc[:, :NS], psv2[:])
        # LayerNorm (g2, b2) into h0v[:, :, 64:128]; center rows; AG halves
        h0e = hv.tile([P, NT, G], dt.bfloat16)
        hc0 = hv.tile([P, NT, G], dt.bfloat16)
        muv = hv.tile([P, NT], dt.float32)
        scrg = hv.tile([P, G], dt.bfloat16)
        for hf in range(2):
            nts = range(hf * NTH, (hf + 1) * NTH)
            nsl = slice(hf * NTH, (hf + 1) * NTH)
            psh = psT.tile([P, NTH, H], dt.bfloat16, tag="pst", name=f"mv{hf}")
            for i, nt in enumerate(nts):
                nc.tensor.transpose(psh[:, i, :], mTc[:, nt * P:(nt + 1) * P],
                                    ident_b[:H, :H])
            drain(cen[:, nsl, :], psh[:])
            nc.vector.tensor_copy(h0v[:, nsl, 0:NF], nf_sb[:, nsl, :])
            nc.vector.tensor_mul(sqf[:, 0:NTH, :], cen[:, nsl, :],
                                 cen[:, nsl, :])
            nc.vector.tensor_reduce(var[:, nsl], sqf[:, 0:NTH, :],
                                    mybir.AxisListType.X, Alu.add)
            nc.scalar.activation(rstd[:, nsl], var[:, nsl], Act.Sqrt,
                                 scale=1.0 / H, bias=epsb[:])
            nc.vector.reciprocal(rstd[:, nsl], rstd[:, nsl])
            for nt in nts:
                nc.vector.scalar_tensor_tensor(mhat[:, nt, :], cen[:, nt, :],
                                               rstd[:, nt:nt + 1], g2m[:],
                                               Alu.mult, Alu.mult)
                nc.vector.tensor_add(h0v[:, nt, NF:G], mhat[:, nt, :], b2m[:])
            nc.vector.tensor_reduce(muv[:, nsl], h0v[:, nsl, :],
                                    mybir.AxisListType.X, Alu.add)
            nc.scalar.mul(muv[:, nsl], muv[:, nsl], 1.0 / G)
            for nt in nts:
                nc.vector.tensor_scalar_sub(hc0[:, nt, :], h0v[:, nt, :],
                                            muv[:, nt:nt + 1])
            nc.sync.dma_start(agv[0][hf][0][:], hc0[:, nsl, :])
            all_gather(agv[0][hf][0], agv[0][hf][1])
        # h0e = (1+eps) * h0_v + b3 (matmul operand); fills the AG gap
        for nt in range(NT):
            nc.vector.tensor_mul(scrg[:], h0v[:, nt, :], epsm[:])
            nc.vector.tensor_add(h0e[:, nt, :], scrg[:], b3m[:])

        def load_hv(ag_pair, name):
            vA = hv.tile([P, C * NTH, G], dt.bfloat16, tag="vA", bufs=2,
                         name=name + "a")
            vB = hv.tile([P, C * NTH, G], dt.bfloat16, tag="vB", bufs=2,
                         name=name + "b")
            nc.sync.dma_start(vA[:].rearrange("p (c e) g -> p c e g", c=C),
                              ag_pair[0][1][:].rearrange(
                                  "(c p) e g -> p c e g", p=P))
            nc.sync.dma_start(vB[:].rearrange("p (c e) g -> p c e g", c=C),
                              ag_pair[1][1][:].rearrange(
                                  "(c p) e g -> p c e g", p=P))
            return vA, vB

        warm(14)
        vA, vB = load_hv(agv[0], "v0")

        # =============== P4: GIN iterations ===============
        zT = hv.tile([G, NS], dt.bfloat16)
        zc = hv.tile([P, NT, G], dt.bfloat16)
        pre = hv.tile([P, NT, G], dt.bfloat16)
        preT = hv.tile([G, NS], dt.bfloat16)
        varz = hv.tile([P, NT], dt.float32)
        rstdz = hv.tile([P, NT], dt.float32)
        sqg = hv.tile([P, NTH, G], dt.float32)
        lnp = hv.tile([P, G], dt.bfloat16)

        hv_final = None
        for t in range(TG):
            psz = psA.tile([G, NS], dt.float32, tag="acc", name=f"zT{t}")
            nv = len(KBA_N)
            for half, vsrc in enumerate((vA, vB)):
                for j in range(nv):
                    nc.tensor.matmul(psz[:], vsrc[:, j, :],
                                     atc2[:, half * nv + j, :],
                                     start=(half == 0 and j == 0),
                                     stop=(half == 1 and j == nv - 1))
            drain(zT[:], psz[:])
            hv_new = hv.tile([P, NT, G], dt.bfloat16, tag="hvnew", bufs=2,
                             name=f"hv_new{t}")
            for hf in range(2):
                nts = range(hf * NTH, (hf + 1) * NTH)
                nsl = slice(hf * NTH, (hf + 1) * NTH)
                psh = psT.tile([P, NTH, G], dt.bfloat16, tag="pst",
                               name=f"zn{t}_{hf}")
                for i, nt in enumerate(nts):
                    nc.tensor.transpose(psh[:, i, :],
                                        zT[:, nt * P:(nt + 1) * P], ident_b[:])
                drain(zc[:, nsl, :], psh[:])
                nc.vector.tensor_mul(sqg[:], zc[:, nsl, :], zc[:, nsl, :])
                nc.vector.tensor_reduce(varz[:, nsl], sqg[:],
                                        mybir.AxisListType.X, Alu.add)
                nc.scalar.activation(rstdz[:, nsl], varz[:, nsl], Act.Sqrt,
                                     scale=1.0 / G, bias=epsb[:])
                nc.vector.reciprocal(rstdz[:, nsl], rstdz[:, nsl])
                psh2 = psT.tile([P, NTH, P], dt.bfloat16, tag="pst",
                                name=f"pT{t}_{hf}")
                for i, nt in enumerate(nts):
                    nc.vector.scalar_tensor_tensor(lnp[:], zc[:, nt, :],
                                                   rstdz[:, nt:nt + 1], g3m[:],
                                                   Alu.mult, Alu.mult)
                    nc.vector.tensor_add(pre[:, nt, :], lnp[:], h0e[:, nt, :])
                    nc.tensor.transpose(psh2[:, i, :], pre[:, nt, :],
                                        ident_b[:])
                drain(preT[:, hf * NTH * P:(hf + 1) * NTH * P]
                      .rearrange("a (e p) -> a e p", p=P), psh2[:])
                psw = psW.tile([P, NTH, G], dt.float32, tag="wps",
                               name=f"wg{t}_{hf}")
                for i, nt in enumerate(nts):
                    nc.tensor.matmul(psw[:, i, :],
                                     preT[:, nt * P:(nt + 1) * P],
                                     wg_sb[:, t, :], start=True, stop=True)
                for i, nt in enumerate(nts):
                    nc.vector.tensor_add(hv_new[:, nt, :], psw[:, i, :],
                                         bgm[:, t * G:t * G + G])
                if t < TG - 1:
                    nc.sync.dma_start(agv[t + 1][hf][0][:], hv_new[:, nsl, :])
                    all_gather(agv[t + 1][hf][0], agv[t + 1][hf][1])
            if t < TG - 1:
                warm(12)
                vA, vB = load_hv(agv[t + 1], f"v{t + 1}")
            else:
                hv_final = hv_new

        # =============== P5: readout + final LayerNorm ===============
        ps_sum = psW.tile([1, G], dt.float32, tag="wps", name="ps_sum")
        for nt in range(NT):
            nc.tensor.matmul(ps_sum[:], ones_g[:], hv_final[:, nt, :],
                             start=(nt == 0), stop=(nt == NT - 1))
        sum_sb = hv.tile([1, G], dt.float32)
        nc.vector.tensor_copy(sum_sb[:], ps_sum[:])
        nc.sync.dma_start(ags_in[:], sum_sb[:])
        if fake_cc:
            nc.sync.dma_start(ags_out[0:1], ags_in[:])
        else:
            nc.gpsimd.collective_compute("AllGather", Alu.bypass,
                                         replica_groups=rg,
                                         ins=[ags_in[:]], outs=[ags_out[:]])
        gall = hv.tile([C, G], dt.float32)
        nc.sync.dma_start(gall[:], ags_out[:])
        ps_g = psW.tile([1, G], dt.float32, tag="wps", name="ps_g")
        nc.tensor.matmul(ps_g[:], ones8f[:], gall[:], start=True, stop=True)
        gsum = hv.tile([1, G], dt.float32)
        nc.vector.tensor_copy(gsum[:], ps_g[:])

        s1 = hv.tile([1, 1], dt.float32)
        nc.vector.tensor_reduce(s1[:], gsum[:], mybir.AxisListType.X, Alu.add)
        nc.scalar.mul(s1[:], s1[:], 1.0 / G)
        cenf = hv.tile([1, G], dt.float32)
        nc.vector.tensor_scalar_sub(cenf[:], gsum[:], s1[:])
        varf = hv.tile([1, 1], dt.float32)
        outf = hv.tile([1, G], dt.float32)
        nc.scalar.activation(outf[:], cenf[:], Act.Square, accum_out=varf[:])
        nc.scalar.activation(varf[:], varf[:], Act.Sqrt, scale=1.0 / G,
                             bias=epsb[:1, :])
        nc.vector.reciprocal(varf[:], varf[:])
        nc.vector.tensor_scalar_mul(cenf[:], cenf[:], varf[:])
        nc.vector.tensor_mul(outf[:], cenf[:], g4_sb[:])
        nc.vector.tensor_add(outf[:], outf[:], b4_sb[:])
        nc.sync.dma_start(out[:], outf[:])

        stack.close()
    nc.compile()
    return nc


_NC_CACHE = {}


def _get_nc(repeat: int = 1, fake_cc: bool = False):
    key = (repeat, fake_cc)
    if key not in _NC_CACHE:
        _NC_CACHE[key] = _build(repeat, fake_cc)
    return _NC_CACHE[key]


def _shard_inputs(inputs):
    f32 = np.float32
    bf16 = ml_dtypes.bfloat16
    f8 = ml_dtypes.float8_e4m3
    ae = np.asarray(inputs["adj_matrix_edges_wo"], f32)
    ane = np.asarray(inputs["atm_dir_edge_adj_matrix"], f32)
    a = np.asarray(inputs["adj_matrix"], f32)
    eal = np.asarray(inputs["edge_aligned_node_features"], f32)
    ef = np.asarray(inputs["dir_edge_features"], f32)
    nf = np.asarray(inputs["node_features"], f32)
    w_init = np.asarray(inputs["W_init"], f32)
    b_init = np.asarray(inputs["b_init"], f32)
    w_pass = np.asarray(inputs["W_pass"], f32)
    w_gin = np.asarray(inputs["W_gin"], f32)
    b_gin = np.asarray(inputs["b_gin"], f32)
    eps = np.asarray(inputs["eps"], f32)
    g1, b1 = np.asarray(inputs["g1"], f32), np.asarray(inputs["b1"], f32)
    g2, b2 = np.asarray(inputs["g2"], f32), np.asarray(inputs["b2"], f32)
    g3, b3 = np.asarray(inputs["g3"], f32), np.asarray(inputs["b3"], f32)
    g4, b4 = np.asarray(inputs["g4"], f32), np.asarray(inputs["b4"], f32)

    # host-side transposes + fp8 cast (0/1 exact in fp8e4)
    aeT = np.ascontiguousarray(ae.T).astype(f8)
    aneT = np.ascontiguousarray(ane.T).astype(f8)
    aT = np.ascontiguousarray(a.T).astype(f8)

    def swizzle(mT, cols, order):
        # [K*P, cols] -> [P, K*cols]: row p holds its k-blocks contiguously
        # (one DMA descriptor per partition), permuted into AG-half order so
        # cache slot i pairs with AG-half state slot i
        k = mT.shape[0] // P
        blocks = mT.reshape(k, P, cols)[np.asarray(order)]
        return np.ascontiguousarray(
            blocks.transpose(1, 0, 2).reshape(P, k * cols))

    # augmented init weights: h0 = relu([eal|ef|1] @ [W_init; b_init])
    waug = np.concatenate([w_init, b_init[None, :]], axis=0).astype(bf16)

    # fold LN1 affine into W_pass: LN(m)@W = mhat@(g1*W) + (b1@W)
    wp = (g1[:, None] * w_pass).transpose(1, 0, 2).reshape(H, TD * H)
    c1 = np.einsum("h,thk->tk", b1, w_pass).reshape(1, TD * H)

    # GIN: fold g3 via DVE; fold feature-centering C_G into W_gin for t<2
    # (so the carried state is always column-centered), keep t=2 plain.
    # b3 rides inside the matmul operand (h0e = (1+eps)*h0_v + b3), so the
    # post-matmul bias is b_gin alone.
    CG = np.eye(G, dtype=f32) - 1.0 / G
    wg_list, bg_list = [], []
    for t in range(TG):
        w_eff = w_gin[t] @ CG if t < TG - 1 else w_gin[t]
        b_eff = b_gin[t] @ CG if t < TG - 1 else b_gin[t]
        wg_list.append(w_eff)
        bg_list.append(b_eff)
    wg = np.stack(wg_list).transpose(1, 0, 2).reshape(G, TG * G)
    bg = np.stack(bg_list).reshape(1, TG * G)

    shared = {
        "waug": np.ascontiguousarray(waug),
        "wp_t": np.ascontiguousarray(wp.astype(bf16)),
        "wg_t": np.ascontiguousarray(wg.astype(bf16)),
        "c1_t": np.ascontiguousarray(c1),
        "bg_t": np.ascontiguousarray(bg),
        "g2_t": g2.reshape(1, H).copy(), "b2_t": b2.reshape(1, H).copy(),
        "g3_t": g3.reshape(1, G).copy(), "b3_t": b3.reshape(1, G).copy(),
        "eps_t": eps.reshape(1, G).copy(),
        "g4_t": g4.reshape(1, G).copy(), "b4_t": b4.reshape(1, G).copy(),
    }
    in_maps = []
    ones_col = np.ones((ES, 1), f32)
    for c in range(C):
        er = slice(c * ES, (c + 1) * ES)
        nr = slice(c * NS, (c + 1) * NS)
        m = dict(shared)
        m["aeT_s"] = swizzle(np.ascontiguousarray(aeT[:, er]), ES, ORD_E)
        m["aneT_s"] = swizzle(np.ascontiguousarray(aneT[:, nr]), NS, ORD_E)
        m["aT_s"] = swizzle(np.ascontiguousarray(aT[:, nr]), NS, ORD_N)
        xaug = np.concatenate([eal[er], ef[er], ones_col], axis=1)  # [ES, 81]
        m["xaugT_s"] = np.ascontiguousarray(xaug.T.astype(bf16))
        m["nf_s"] = np.ascontiguousarray(
            nf[nr].reshape(NT, P, NF).transpose(1, 0, 2).reshape(P, NT * NF)
            .astype(bf16))
        in_maps.append(m)
    return in_maps


def run(inputs, **spmd_kwargs):
    """Run on hardware; returns (output, BassKernelResults)."""
    nc = _get_nc()
    in_maps = _shard_inputs(inputs)
    res = run_bass_kernel_spmd(nc, in_maps, core_ids=list(range(C)), **spmd_kwargs)
    return res.results[0]["out"], res


def kernel(**inputs) -> np.ndarray:
    out, _ = run(inputs)
    return np.ascontiguousarray(out, dtype=np.float32)
